# revision 19
# baseline (speedup 1.0000x reference)
"""CrossCosineEmbeddingLoss kernel for 8 trn2 NeuronCores (v9).

loss = mean over all (i,j) of: 1 - cos(x_i, y_j) if i==j else relu(cos(x_i, y_j))
     = [ sum_ij relu(sim) + sum_i (1 - sim_ii - relu(sim_ii)) ] / n^2

Sharding (4x2 grid): core c=(a,b): x rows [2048a, ..+2048), y rows
[4096b, ..+4096).  32 j-tiles x 2048 i-cols per core; diag terms on every
core vs its own x rows (host counts them from b==0 cores only).

Engines execute their compiled streams IN ORDER, so emission order is
runtime order per engine:
  - drains accumulate UNscaled sum_i relu(z); 1/||y_j|| applied post-hoc
    (relu commutes with positive scales) -> norms off the critical path
  - PSUM = 4 rotating [128,1024] buffers (2 banks each): two half-drains
    per j-tile pipeline 4-deep, keeping ACT (34 halves) and DVE (30)
    saturated
  - side work (ny2 reduces, diag reduces, rsqrt chains) is slotted into
    the drain stream at points where its GpSimd producers are done
  - GpSimd: y squares + half-folds, diag products + folds
  - p-major DMA layouts; batched XBAR transposes on the sync queue
Host combines [128,2] per-core partials.
"""

import numpy as np

import concourse.bacc as bacc
import concourse.bass as bass
import concourse.tile as tile
from concourse import mybir
from concourse.bass_utils import run_bass_kernel_spmd

N, D = 8192, 128
NCORES = 8
GA, GB = 4, 2
XS = N // GA              # 2048 x rows per core
YS = N // GB              # 4096 y rows per core
XT = XS // 128            # 16 x-tiles
JT = YS // 128            # 32 j-tiles
NH = 2 * JT               # 64 half-tile drains
XC = 4                    # x chunks (4 tiles each)
XCT = XT // XC
YG = 4                    # y groups (8 tiles each)
GT = JT // YG

f32 = mybir.dt.float32
bf16 = mybir.dt.bfloat16
AF = mybir.ActivationFunctionType
ALU = mybir.AluOpType
AX = mybir.AxisListType

# drain split ACT : DVE = 34 : 30 (ACT ~1.24us, DVE ~1.18us + side work);
# phase chosen so h=0 goes to ACT (free earliest at ramp end)
ACT_DRAIN = {h for h in range(NH) if (h * 34) % NH < 34}

_CACHE = {}


def _build():
    if "nc" in _CACHE:
        return _CACHE["nc"]
    nc = bacc.Bacc("TRN2", target_bir_lowering=False, debug=False,
                   num_devices=NCORES)
    xs_d = nc.dram_tensor("xs", [XS, D], f32, kind="ExternalInput")
    ys_d = nc.dram_tensor("ys", [YS, D], f32, kind="ExternalInput")
    yd_d = nc.dram_tensor("yd", [XS, D], f32, kind="ExternalInput")
    out_d = nc.dram_tensor("out", [128, 2], f32, kind="ExternalOutput")

    with tile.TileContext(nc) as tc:
        with (
            tc.tile_pool(name="singles", bufs=1) as singles,
            tc.tile_pool(name="yload", bufs=4) as yload,
            tc.tile_pool(name="scr", bufs=2) as scr,
        ):
            xnat = singles.tile([128, XT, 128], f32)
            xbf = singles.tile([128, XT, 128], bf16)
            xhatT = singles.tile([128, XT, 128], bf16)
            yT = singles.tile([128, JT, 128], bf16)
            ybfs = [singles.tile([128, GT, 128], bf16, name=f"ybf{g}")
                    for g in range(YG)]
            sqfs = [singles.tile([128, GT, 64], f32, name=f"sqf{g}")
                    for g in range(YG)]
            ydn = singles.tile([128, XT, 128], f32)
            nx2 = singles.tile([128, XT], f32)
            rnx = singles.tile([128, XT], f32)
            ny2 = singles.tile([128, JT], f32)
            rny = singles.tile([128, JT], f32)
            t1x = singles.tile([128, XT], f32)
            t2y = singles.tile([128, JT], f32)
            racc = singles.tile([128, JT, 2], f32)
            rsc = singles.tile([128, JT, 2], f32)
            prodd = singles.tile([128, XT, 128], bf16)
            sqd = singles.tile([128, XT, 128], bf16)
            tmp1 = singles.tile([128, XT, 64], f32)
            tmp2 = singles.tile([128, XT, 64], f32)
            pfold = singles.tile([128, XT, 32], f32)
            sfold = singles.tile([128, XT, 32], f32)
            d2 = singles.tile([128, XT], f32)
            nyd2 = singles.tile([128, XT], f32)
            rnyd = singles.tile([128, XT], f32)
            simd = singles.tile([128, XT], f32)
            relud = singles.tile([128, XT], f32)
            sd_scr = singles.tile([128, XT], f32)
            outsb = singles.tile([128, 2], f32)
            warm = singles.tile([128, 2], f32)

            # warmups
            nc.vector.memset(warm[:, 0:1], 1.0)
            nc.vector.reciprocal(warm[:, 1:2], warm[:, 0:1])
            nc.scalar.sqrt(warm[:, 0:1], warm[:, 1:2])

            # ---- ALL loads issued first on the sync queue (transposes
            # block the sync engine, so they must come after every load)
            ynats = []
            for g in range(YG):
                yv = ys_d[1024 * g:1024 * (g + 1)].rearrange(
                    "(p t) d -> p t d", t=GT)
                ynat = yload.tile([128, GT, 128], f32, tag="ynat")
                nc.sync.dma_start(out=ynat[:], in_=yv)
                ynats.append(ynat)
            for c in range(XC):
                ts = slice(XCT * c, XCT * (c + 1))
                xv = xs_d[512 * c:512 * (c + 1)].rearrange(
                    "(p t) d -> p t d", t=XCT)
                nc.sync.dma_start(out=xnat[:, ts, :], in_=xv)

            def y_cast(g, act):
                # ACT has its own SBUF port (no gpsimd contention); DVE
                # takes the late groups after its x work
                if act:
                    nc.scalar.copy(out=ybfs[g][:], in_=ynats[g][:])
                else:
                    nc.vector.tensor_copy(out=ybfs[g][:], in_=ynats[g][:])

            def y_sq(g):
                ybf = ybfs[g]
                sq = scr.tile([128, GT, 128], bf16, tag="ysq")
                nc.gpsimd.tensor_tensor(
                    out=sq[:], in0=ybf[:], in1=ybf[:], op=ALU.mult)
                nc.gpsimd.tensor_tensor(
                    out=sqfs[g][:], in0=sq[:, :, 0:64], in1=sq[:, :, 64:128],
                    op=ALU.add)

            def x_chunk(c):
                ts = slice(XCT * c, XCT * (c + 1))
                sq = scr.tile([128, XCT, 128], f32, tag="xsq")
                nc.vector.tensor_tensor(
                    out=sq[:], in0=xnat[:, ts, :], in1=xnat[:, ts, :],
                    op=ALU.mult)
                nc.vector.tensor_reduce(
                    out=nx2[:, ts], in_=sq[:], axis=AX.X, op=ALU.add)
                nc.vector.reciprocal(t1x[:, ts], nx2[:, ts])
                nc.scalar.sqrt(rnx[:, ts], t1x[:, ts])
                for k in range(XCT):
                    t = XCT * c + k
                    nc.scalar.activation(
                        xbf[:, t, :], xnat[:, t, :], AF.Copy,
                        scale=rnx[:, t:t + 1])

            # casts 0/1 on ACT (own port), 2/3 on DVE after its x chunks
            y_cast(0, act=True)
            y_cast(1, act=True)
            x_chunk(0)
            x_chunk(1)
            x_chunk(2)
            x_chunk(3)
            y_cast(2, act=False)
            y_cast(3, act=False)
            y_sq(0)
            y_sq(1)
            y_sq(2)
            y_sq(3)

            # transposes on sync, in producer-completion order
            def y_xpose(g):
                ts = slice(GT * g, GT * (g + 1))
                nc.sync.dma_start(out=yT[:, ts, :], in_=ybfs[g][:],
                                  transpose=True)

            def x_xpose(c):
                ts = slice(XCT * c, XCT * (c + 1))
                nc.sync.dma_start(out=xhatT[:, ts, :], in_=xbf[:, ts, :],
                                  transpose=True)

            y_xpose(0)
            x_xpose(0)
            x_xpose(1)
            y_xpose(1)
            x_xpose(2)
            x_xpose(3)
            y_xpose(2)
            y_xpose(3)

            # diag loads last (only gpsimd consumes them, mid-window)
            for c in range(XC):
                ydv = yd_d[512 * c:512 * (c + 1)].rearrange(
                    "(p t) d -> p t d", t=XCT)
                nc.sync.dma_start(out=ydn[:, XCT * c:XCT * (c + 1), :],
                                  in_=ydv)
            nc.gpsimd.tensor_tensor(
                out=prodd[:], in0=xnat[:], in1=ydn[:], op=ALU.mult)
            nc.gpsimd.tensor_tensor(
                out=sqd[:], in0=ydn[:], in1=ydn[:], op=ALU.mult)
            nc.gpsimd.tensor_tensor(
                out=tmp1[:], in0=prodd[:, :, 0:64], in1=prodd[:, :, 64:128],
                op=ALU.add)
            nc.gpsimd.tensor_tensor(
                out=tmp2[:], in0=sqd[:, :, 0:64], in1=sqd[:, :, 64:128],
                op=ALU.add)
            nc.gpsimd.tensor_tensor(
                out=pfold[:], in0=tmp1[:, :, 0:32], in1=tmp1[:, :, 32:64],
                op=ALU.add)
            nc.gpsimd.tensor_tensor(
                out=sfold[:], in0=tmp2[:, :, 0:32], in1=tmp2[:, :, 32:64],
                op=ALU.add)

            # side work slotted into the drain streams (producers done by
            # then: gpsimd folds finish ~mid-window)
            def side_work(t):
                if t == 13:
                    nc.vector.tensor_reduce(
                        out=ny2[:, 0:GT], in_=sqfs[0][:], axis=AX.X,
                        op=ALU.add)
                    nc.vector.tensor_reduce(
                        out=ny2[:, GT:2 * GT], in_=sqfs[1][:], axis=AX.X,
                        op=ALU.add)
                elif t == 19:
                    nc.vector.tensor_reduce(
                        out=ny2[:, 2 * GT:3 * GT], in_=sqfs[2][:], axis=AX.X,
                        op=ALU.add)
                    nc.vector.tensor_reduce(
                        out=ny2[:, 3 * GT:4 * GT], in_=sqfs[3][:], axis=AX.X,
                        op=ALU.add)
                    nc.vector.reciprocal(t2y[:], ny2[:])
                elif t == 22:
                    nc.scalar.sqrt(rny[:], t2y[:])
                elif t == 27:
                    nc.vector.tensor_reduce(
                        out=d2[:], in_=pfold[:], axis=AX.X, op=ALU.add)
                    nc.vector.tensor_reduce(
                        out=nyd2[:], in_=sfold[:], axis=AX.X, op=ALU.add)
                    nc.vector.reciprocal(sd_scr[:], nyd2[:])
                elif t == 28:
                    nc.scalar.sqrt(rnyd[:], sd_scr[:])
                elif t == 30:
                    nc.vector.tensor_tensor(
                        out=simd[:], in0=d2[:], in1=rnx[:], op=ALU.mult)
                    nc.vector.tensor_tensor(
                        out=simd[:], in0=simd[:], in1=rnyd[:], op=ALU.mult)
                    nc.scalar.activation(relud[:], simd[:], AF.Relu)

            # ---- main loop: 2 half-tiles per j-tile, 4-deep PSUM pipeline
            rhs = xhatT[:].rearrange("p a b -> p (a b)")
            with tc.tile_pool(name="mpsum", bufs=4, space="PSUM") as mpsum:
                for t in range(JT):
                    lhsT = yT[:, t, :]
                    for e in range(2):
                        h = 2 * t + e
                        ps = mpsum.tile([128, 1024], f32, tag="mp")
                        for q in range(2):
                            col = 1024 * e + 512 * q
                            nc.tensor.matmul(
                                ps[:, 512 * q:512 * (q + 1)], lhsT,
                                rhs[:, col:col + 512])
                        if h in ACT_DRAIN:
                            nc.scalar.activation(
                                ps[:], ps[:], AF.Relu,
                                accum_out=racc[:, t, e:e + 1])
                        else:
                            nc.vector.tensor_scalar(
                                out=ps[:], in0=ps[:], scalar1=0.0,
                                scalar2=None, op0=ALU.max, op1=ALU.add,
                                accum_out=racc[:, t, e:e + 1])
                    side_work(t)

            # ---- tail: diag accum, scale accumulators by rny, reduce, out
            nc.vector.scalar_tensor_tensor(
                out=sd_scr[:], in0=simd[:], scalar=1.0, in1=relud[:],
                op0=ALU.mult, op1=ALU.add, accum_out=outsb[:, 1:2])
            nc.vector.tensor_tensor(
                out=rsc[:], in0=racc[:],
                in1=rny[:].unsqueeze(2).broadcast_to([128, JT, 2]),
                op=ALU.mult)
            nc.vector.tensor_reduce(
                out=outsb[:, 0:1], in_=rsc[:], axis=AX.XY, op=ALU.add)
            nc.sync.dma_start(out=out_d[:], in_=outsb[:])

    nc.compile()
    _CACHE["nc"] = nc
    return nc


def _in_maps(x, y):
    maps = []
    for c in range(NCORES):
        a, b = c // GB, c % GB
        maps.append({
            "xs": np.ascontiguousarray(x[XS * a:XS * (a + 1)]),
            "ys": np.ascontiguousarray(y[YS * b:YS * (b + 1)]),
            "yd": np.ascontiguousarray(y[XS * a:XS * (a + 1)]),
        })
    return maps


def _combine(results):
    total = 0.0
    for c in range(NCORES):
        o = results[c]["out"].astype(np.float64)
        total += o[:, 0].sum()
        if c % GB == 0:
            total += XS - o[:, 1].sum()
    return np.float32(total / (float(N) * float(N)))


def _run(x, y, trace=False):
    nc = _build()
    res = run_bass_kernel_spmd(nc, _in_maps(x, y), list(range(NCORES)),
                               trace=trace)
    return _combine(res.results), res


def kernel(x, y):
    x = np.asarray(x, dtype=np.float32)
    y = np.asarray(y, dtype=np.float32)
    loss, _ = _run(x, y, trace=False)
    return loss


# revision 20
# speedup vs baseline: 1.0622x; 1.0622x over previous
"""CrossCosineEmbeddingLoss kernel for 8 trn2 NeuronCores (v8).

loss = mean over all (i,j) of: 1 - cos(x_i, y_j) if i==j else relu(cos(x_i, y_j))
     = [ sum_ij relu(sim) + sum_i (1 - sim_ii - relu(sim_ii)) ] / n^2

Sharding (4x2 grid): core c=(a,b): x rows [2048a, ..+2048), y rows
[4096b, ..+4096).  32 j-tiles x 2048 i-cols per core; diag terms on every
core vs its own x rows (host counts them from b==0 cores only).

Key structure:
  - drains accumulate UNscaled sum_i relu(z); 1/||y_j|| applied post-hoc
    to the [128,64] accumulator (relu commutes with positive scales), so
    norms are fully off the critical path.
  - PSUM as 4 rotating buffers of [128,1024] (2 banks each): per j-tile
    two half-drains pipeline 4-deep so ACT/DVE stay saturated (2-buffer
    [128,2048] serialized MM+drain per buffer and left both engines ~35%
    idle).
  - bf16 matmuls: xhat normalized+cast via ACT per-tile scale-copy; y cast
    on DVE; all transposes via batched DMA-XBAR (dma_start transpose=True)
  - p-major DMA layouts (>=2KB contiguous descriptors)
  - GpSimd: y squares + half-folds, diag products + folds (idle engine
    absorbs elementwise work; DVE only does short final reduces)
Host combines [128,2] per-core partials.
"""

import numpy as np

import concourse.bacc as bacc
import concourse.bass as bass
import concourse.tile as tile
from concourse import mybir
from concourse.bass_utils import run_bass_kernel_spmd

N, D = 8192, 128
NCORES = 8
GA, GB = 4, 2
XS = N // GA              # 2048 x rows per core
YS = N // GB              # 4096 y rows per core
XT = XS // 128            # 16 x-tiles
JT = YS // 128            # 32 j-tiles
NH = 2 * JT               # 64 half-tile drains
XC = 4                    # x chunks (4 tiles each)
XCT = XT // XC
YG = 4                    # y groups (8 tiles each)
GT = JT // YG

f32 = mybir.dt.float32
bf16 = mybir.dt.bfloat16
AF = mybir.ActivationFunctionType
ALU = mybir.AluOpType
AX = mybir.AxisListType

# drain split per half-tile, ACT : DVE ~ 1 : 1
ACT_DRAIN = {h for h in range(NH) if h % 2 == 0}

_CACHE = {}


def _build():
    if "nc" in _CACHE:
        return _CACHE["nc"]
    nc = bacc.Bacc("TRN2", target_bir_lowering=False, debug=False,
                   num_devices=NCORES)
    xs_d = nc.dram_tensor("xs", [XS, D], f32, kind="ExternalInput")
    ys_d = nc.dram_tensor("ys", [YS, D], f32, kind="ExternalInput")
    yd_d = nc.dram_tensor("yd", [XS, D], f32, kind="ExternalInput")
    out_d = nc.dram_tensor("out", [128, 2], f32, kind="ExternalOutput")

    with tile.TileContext(nc) as tc:
        with (
            tc.tile_pool(name="singles", bufs=1) as singles,
            tc.tile_pool(name="yload", bufs=3) as yload,
            tc.tile_pool(name="scr", bufs=2) as scr,
        ):
            xnat = singles.tile([128, XT, 128], f32)
            xbf = singles.tile([128, XT, 128], bf16)
            xhatT = singles.tile([128, XT, 128], bf16)
            yT = singles.tile([128, JT, 128], bf16)
            ybfs = [singles.tile([128, GT, 128], bf16, name=f"ybf{g}")
                    for g in range(YG)]
            ydn = singles.tile([128, XT, 128], f32)
            nx2 = singles.tile([128, XT], f32)
            rnx = singles.tile([128, XT], f32)
            ny2 = singles.tile([128, JT], f32)
            rny = singles.tile([128, JT], f32)
            t1x = singles.tile([128, XT], f32)
            t2y = singles.tile([128, JT], f32)
            racc = singles.tile([128, JT, 2], f32)
            rsc = singles.tile([128, JT, 2], f32)
            prodd = singles.tile([128, XT, 128], bf16)
            sqd = singles.tile([128, XT, 128], bf16)
            tmp1 = singles.tile([128, XT, 64], f32)
            tmp2 = singles.tile([128, XT, 64], f32)
            pfold = singles.tile([128, XT, 32], f32)
            sfold = singles.tile([128, XT, 32], f32)
            d2 = singles.tile([128, XT], f32)
            nyd2 = singles.tile([128, XT], f32)
            rnyd = singles.tile([128, XT], f32)
            simd = singles.tile([128, XT], f32)
            relud = singles.tile([128, XT], f32)
            sd_scr = singles.tile([128, XT], f32)
            outsb = singles.tile([128, 2], f32)
            warm = singles.tile([128, 2], f32)

            # warmups: DVE first-op penalty + ACT table set
            nc.vector.memset(warm[:, 0:1], 1.0)
            nc.vector.reciprocal(warm[:, 1:2], warm[:, 0:1])
            nc.scalar.sqrt(warm[:, 0:1], warm[:, 1:2])

            def y_group(g):
                ts = slice(GT * g, GT * (g + 1))
                yv = ys_d[1024 * g:1024 * (g + 1)].rearrange(
                    "(p t) d -> p t d", t=GT)
                ynat = yload.tile([128, GT, 128], f32, tag="ynat")
                nc.sync.dma_start(out=ynat[:], in_=yv)
                ybf = ybfs[g]
                nc.vector.tensor_copy(out=ybf[:], in_=ynat[:])
                with tc.high_priority():
                    nc.sync.dma_start(out=yT[:, ts, :], in_=ybf[:],
                                      transpose=True)
                sq = scr.tile([128, GT, 128], bf16, tag="ysq")
                nc.gpsimd.tensor_tensor(
                    out=sq[:], in0=ybf[:], in1=ybf[:], op=ALU.mult)
                sqf = scr.tile([128, GT, 64], f32, tag="ysqf")
                nc.gpsimd.tensor_tensor(
                    out=sqf[:], in0=sq[:, :, 0:64], in1=sq[:, :, 64:128],
                    op=ALU.add)
                nc.vector.tensor_reduce(
                    out=ny2[:, ts], in_=sqf[:], axis=AX.X, op=ALU.add)

            def x_chunk(c):
                ts = slice(XCT * c, XCT * (c + 1))
                xv = xs_d[512 * c:512 * (c + 1)].rearrange(
                    "(p t) d -> p t d", t=XCT)
                nc.sync.dma_start(out=xnat[:, ts, :], in_=xv)
                sq = scr.tile([128, XCT, 128], f32, tag="xsq")
                nc.vector.tensor_tensor(
                    out=sq[:], in0=xnat[:, ts, :], in1=xnat[:, ts, :],
                    op=ALU.mult)
                nc.vector.tensor_reduce(
                    out=nx2[:, ts], in_=sq[:], axis=AX.X, op=ALU.add)
                nc.vector.reciprocal(t1x[:, ts], nx2[:, ts])
                nc.scalar.sqrt(rnx[:, ts], t1x[:, ts])
                for k in range(XCT):
                    t = XCT * c + k
                    nc.scalar.activation(
                        xbf[:, t, :], xnat[:, t, :], AF.Copy,
                        scale=rnx[:, t:t + 1])
                with tc.high_priority():
                    nc.sync.dma_start(out=xhatT[:, ts, :],
                                      in_=xbf[:, ts, :], transpose=True)

            y_group(0)
            x_chunk(0)
            x_chunk(1)
            y_group(1)
            x_chunk(2)
            x_chunk(3)
            y_group(2)
            y_group(3)

            # diag inputs (gpsimd work queued behind y squares)
            for c in range(XC):
                ydv = yd_d[512 * c:512 * (c + 1)].rearrange(
                    "(p t) d -> p t d", t=XCT)
                nc.sync.dma_start(out=ydn[:, XCT * c:XCT * (c + 1), :],
                                  in_=ydv)
            nc.gpsimd.tensor_tensor(
                out=prodd[:], in0=xnat[:], in1=ydn[:], op=ALU.mult)
            nc.gpsimd.tensor_tensor(
                out=sqd[:], in0=ydn[:], in1=ydn[:], op=ALU.mult)
            nc.gpsimd.tensor_tensor(
                out=tmp1[:], in0=prodd[:, :, 0:64], in1=prodd[:, :, 64:128],
                op=ALU.add)
            nc.gpsimd.tensor_tensor(
                out=tmp2[:], in0=sqd[:, :, 0:64], in1=sqd[:, :, 64:128],
                op=ALU.add)
            nc.gpsimd.tensor_tensor(
                out=pfold[:], in0=tmp1[:, :, 0:32], in1=tmp1[:, :, 32:64],
                op=ALU.add)
            nc.gpsimd.tensor_tensor(
                out=sfold[:], in0=tmp2[:, :, 0:32], in1=tmp2[:, :, 32:64],
                op=ALU.add)

            # ---- norms for y (anytime before tail)
            nc.vector.reciprocal(t2y[:], ny2[:])
            nc.scalar.sqrt(rny[:], t2y[:])

            # ---- main loop: 2 half-tiles per j-tile, 4-deep PSUM pipeline
            rhs = xhatT[:].rearrange("p a b -> p (a b)")
            with tc.tile_pool(name="mpsum", bufs=4, space="PSUM") as mpsum:
                for t in range(JT):
                    lhsT = yT[:, t, :]
                    for e in range(2):
                        h = 2 * t + e
                        ps = mpsum.tile([128, 1024], f32, tag="mp")
                        for q in range(2):
                            col = 1024 * e + 512 * q
                            nc.tensor.matmul(
                                ps[:, 512 * q:512 * (q + 1)], lhsT,
                                rhs[:, col:col + 512])
                        if h in ACT_DRAIN:
                            nc.scalar.activation(
                                ps[:], ps[:], AF.Relu,
                                accum_out=racc[:, t, e:e + 1])
                        else:
                            nc.vector.tensor_scalar(
                                out=ps[:], in0=ps[:], scalar1=0.0,
                                scalar2=None, op0=ALU.max, op1=ALU.add,
                                accum_out=racc[:, t, e:e + 1])

            # ---- diag small ops
            nc.vector.tensor_reduce(
                out=d2[:], in_=pfold[:], axis=AX.X, op=ALU.add)
            nc.vector.tensor_reduce(
                out=nyd2[:], in_=sfold[:], axis=AX.X, op=ALU.add)
            nc.vector.reciprocal(sd_scr[:], nyd2[:])
            nc.scalar.sqrt(rnyd[:], sd_scr[:])
            nc.vector.tensor_tensor(out=simd[:], in0=d2[:], in1=rnx[:],
                                    op=ALU.mult)
            nc.vector.tensor_tensor(out=simd[:], in0=simd[:], in1=rnyd[:],
                                    op=ALU.mult)
            nc.scalar.activation(relud[:], simd[:], AF.Relu)
            nc.vector.scalar_tensor_tensor(
                out=sd_scr[:], in0=simd[:], scalar=1.0, in1=relud[:],
                op0=ALU.mult, op1=ALU.add, accum_out=outsb[:, 1:2])

            # ---- tail: scale accumulators by rny, reduce, ship out
            nc.vector.tensor_tensor(
                out=rsc[:], in0=racc[:],
                in1=rny[:].unsqueeze(2).broadcast_to([128, JT, 2]),
                op=ALU.mult)
            nc.vector.tensor_reduce(
                out=outsb[:, 0:1], in_=rsc[:], axis=AX.XY, op=ALU.add)
            nc.sync.dma_start(out=out_d[:], in_=outsb[:])

    nc.compile()
    _CACHE["nc"] = nc
    return nc


def _in_maps(x, y):
    maps = []
    for c in range(NCORES):
        a, b = c // GB, c % GB
        maps.append({
            "xs": np.ascontiguousarray(x[XS * a:XS * (a + 1)]),
            "ys": np.ascontiguousarray(y[YS * b:YS * (b + 1)]),
            "yd": np.ascontiguousarray(y[XS * a:XS * (a + 1)]),
        })
    return maps


def _combine(results):
    total = 0.0
    for c in range(NCORES):
        o = results[c]["out"].astype(np.float64)
        total += o[:, 0].sum()
        if c % GB == 0:
            total += XS - o[:, 1].sum()
    return np.float32(total / (float(N) * float(N)))


def _run(x, y, trace=False):
    nc = _build()
    res = run_bass_kernel_spmd(nc, _in_maps(x, y), list(range(NCORES)),
                               trace=trace)
    return _combine(res.results), res


def kernel(x, y):
    x = np.asarray(x, dtype=np.float32)
    y = np.asarray(y, dtype=np.float32)
    loss, _ = _run(x, y, trace=False)
    return loss
